# revision 74
# baseline (speedup 1.0000x reference)
"""Trainium2 Bass kernel for nn_Attention_decoder (conv + linear-attention + convT block).

Math refactoring (validated vs reference):
  - All BatchNorms folded into weights/biases (eval mode, affine).
  - No softmax => the two N^2 einsums collapse by associativity:
      m1 = vf^T @ (qf @ k) = (vf^T qf) @ k = A @ k,  A is 128x128.
  - out-proj and out1 folded:  xo = (Wo1a@out_w/16 @ A) @ k + Wo1b @ x_conv + bias.
  - ConvTranspose2d(k=3,s=2,p=1,op=1) decomposed into 4 parity sub-convs.

Sharding: 8 cores = 4 batches x 2 row-halves. The attention Gram matrix
A = vf^T qf is spatially global (all 64 rows); each core owns 32 rows.
Schedule (cross-core exchange):
  - Gram split: S_cc = each core's first 32-D window rows (exchanged with
    the pair partner) + S_dup = the last D rows of BOTH halves, computed
    locally by both cores (own tail from the window conv + a D-row strip
    of the partner's tail conv'd redundantly).
  - The S_cc partial is exchanged pairwise via SWDGE remote_dma_broadcast
    (SBUF -> peer SBUF, relative dest drid=0/dtpb=1, ~2us) instead of a
    ncfw collective_compute AllReduce (~35-45us trigger-to-usable). The
    send trigger is gated on the kernel-entry barrier (a 1-byte prelude
    AllGather on the CC stream, done ~+55us, concurrent with compute) so
    the peer has cleared its sems before the remote sem-inc arrives; both
    that wait and the recv-sem wait are spliced in AFTER Tile scheduling
    (the Tile sim cannot see remote/prelude increments and would deadlock).
  - After the exchange: A = own S_cc + peer S_cc + local dup Gram -> QT,
    attn half of xo, and the transposed conv.
Dtypes: conv1/convT weights+inputs and the whole mid-section (xb, qv, k,
xconv, QT) are bf16 (1 cycle/row matmuls, 4x cheaper LDWEIGHTS than f32r,
half the DMA bytes); PSUM accumulation stays fp32; the exchange payload
and QT accumulation stay f32r. rel_err ~5e-3 vs the 2e-2 gate.
NOTE: D=4/D=6 and some DMA-queue replans hang the NEFF (schedule-sensitive
race somewhere around the exchange) -- D=8 with this exact load plan is
stable across runs. Perturb with care and re-verify on HW.
"""

import os
import sys

for _p in ("/opt/trn_rl_repo", "/root/.axon_site/_ro/trn_rl_repo"):
    if os.path.isdir(_p) and _p not in sys.path:
        sys.path.insert(0, _p)

import ml_dtypes
import numpy as np

import concourse.bass as bass
import concourse.mybir as mybir
import concourse.tile as tile
from concourse import bacc
from concourse.bass_utils import run_bass_kernel_spmd

EPS = 1e-5
B, C, H, W = 4, 256, 64, 64
CH = 128  # attention head dim
P = 128   # partitions
F32 = mybir.dt.float32
F32R = mybir.dt.float32r
BF16 = mybir.dt.bfloat16

D = 8          # dup strip rows per half (exchange-latency cover)
WIN_IN = 35    # input rows for the window conv (33 out rows)
WIN_OUT = 33   # window out rows: 32 half rows + 1 halo
STRIP_IN = D + 2
WPAD = 66      # horizontally padded width


def _rowchunks(r0, r1, maxr=7):
    out = []
    while r0 < r1:
        n = min(maxr, r1 - r0)
        out.append((r0, n))
        r0 += n
    return out


# conv chunking (rows): pass-1 = S_cc rows [0, 32-D); tail = [32-D, 33)
# NB: D in {0, 4, 6} hangs the NEFF on HW regardless of tail or strip
# chunking (both the 2-tail-chunk and 2-strip-chunk theories were tested
# and falsified). Only D=8 and D=12 are known-stable; D=8 is faster. The
# D=0 path below would save ~8.5us if the hang is ever root-caused.
CC_CHUNKS = _rowchunks(0, 32 - D)
TAIL_CHUNKS = _rowchunks(32 - D, 33)
STRIP_CHUNKS = _rowchunks(0, D)
N_G1 = (32 - D) * 64 // 128          # Gram chunks before the exchange
G2_OWN_COLS = [(32 - D) * 64 + 128 * i for i in range(D // 2)]
G2_STRIP_COLS = [128 * i for i in range(D // 2)]
K_CHUNKS = [(0, 7), (7, 7), (14, 7), (21, 6), (27, 6)]
XOP_CHUNKS = [(0, 7), (7, 7), (14, 7), (21, 6), (27, 6)]   # x_conv partials
XO_CHUNKS = [(0, 7), (7, 7), (14, 7), (21, 6), (27, 6)]  # attn+merge

# convT parity grid taps: (r, s) -> [(ky, kx, da, db), ...]
CT_TAPS = {
    (0, 0): [(1, 1, 0, 0)],
    (0, 1): [(1, 0, 0, 1), (1, 2, 0, 0)],
    (1, 0): [(0, 1, 1, 0), (2, 1, 0, 0)],
    (1, 1): [(0, 0, 1, 1), (0, 2, 1, 0), (2, 0, 0, 1), (2, 2, 0, 0)],
}

LAST_EXEC_TIME_NS = None
LAST_PROFILE = None

_CACHE = {}


def _fold_bn(g, b, m, v):
    s = g / np.sqrt(v + EPS)
    return s.astype(np.float64), (b - m * s).astype(np.float64)


def _prep_weights(inp):
    """Host-side BN folding + layout prep. Returns dict of shared per-core arrays."""
    f8 = lambda a: np.asarray(a, np.float64)
    s1, t1 = _fold_bn(f8(inp["bn1_g"]), f8(inp["bn1_b"]), f8(inp["bn1_m"]), f8(inp["bn1_v"]))
    W1 = f8(inp["conv_w"]) * s1[:, None, None, None]          # (co, ci, ky, kx)
    B1 = s1 * f8(inp["conv_b"]) + t1                          # (256,)
    sq, tq = _fold_bn(f8(inp["qbn_g"]), f8(inp["qbn_b"]), f8(inp["qbn_m"]), f8(inp["qbn_v"]))
    Wq = f8(inp["q_w"]) * sq[:, None]
    Bq = sq * f8(inp["q_b"]) + tq
    sk, tk = _fold_bn(f8(inp["kbn_g"]), f8(inp["kbn_b"]), f8(inp["kbn_m"]), f8(inp["kbn_v"]))
    Wk = f8(inp["k_w"]) * sk[:, None]
    Bk = sk * f8(inp["k_b"]) + tk
    Wv = f8(inp["v_w"])
    Bv = f8(inp["v_b"])
    so, to = _fold_bn(f8(inp["obn_g"]), f8(inp["obn_b"]), f8(inp["obn_m"]), f8(inp["obn_v"]))
    Wo1 = f8(inp["out1_w"]) * so[:, None]                     # (256, 512)
    Bo1 = so * f8(inp["out1_b"]) + to
    Wo1a, Wo1b = Wo1[:, :C], Wo1[:, C:]
    Rm = Wo1a @ f8(inp["out_w"]) / 16.0                       # (256, 128)
    bias_xo = Bo1 + Wo1a @ f8(inp["out_b"])                   # (256,)
    trw = f8(inp["tr_w"])                                     # (ci, co, ky, kx)
    trb = f8(inp["tr_b"])

    d = {}
    # conv1 lhsT: w1t[a, o, t*2 + i, b] = W1[o*128+b, i*128+a, ky, kx]
    w1t = np.zeros((P, 2, 18, P), np.float32)
    trt = np.zeros((P, 2, 18, P), np.float32)
    for t in range(9):
        ky, kx = divmod(t, 3)
        for i in range(2):
            for o in range(2):
                idx = t * 2 + i
                w1t[:, o, idx, :] = W1[o*P:(o+1)*P, i*P:(i+1)*P, ky, kx].T
                trt[:, o, idx, :] = trw[i*P:(i+1)*P, o*P:(o+1)*P, ky, kx]
    d["w1t"] = w1t.astype(ml_dtypes.bfloat16)
    d["trt"] = trt.astype(ml_dtypes.bfloat16)
    d["b1"] = np.ascontiguousarray(B1.reshape(2, P).T.astype(np.float32))      # (128, 2)
    d["trb"] = np.ascontiguousarray(trb.reshape(2, P).T.astype(np.float32))    # (128, 2)
    # qv moving weights: qvt[a, i, 0:128]=Wq[c, i*128+a]; [...,128:256]=Wv
    qvt = np.zeros((P, 2, 2 * CH), np.float32)
    for i in range(2):
        qvt[:, i, :CH] = Wq[:, i*P:(i+1)*P].T
        qvt[:, i, CH:] = Wv[:, i*P:(i+1)*P].T
    d["qvt"] = qvt.astype(ml_dtypes.bfloat16)
    d["qvbias"] = np.broadcast_to(
        np.concatenate([Bq, Bv]).astype(np.float32)[None, :], (P, 2 * CH)
    ).copy()
    wkt = np.zeros((P, 2, CH), np.float32)
    for i in range(2):
        wkt[:, i, :] = Wk[:, i*P:(i+1)*P].T
    d["wkt"] = wkt.astype(ml_dtypes.bfloat16)
    d["bk"] = Bk.astype(np.float32).reshape(P, 1)  # packed into smalls below
    d["rt"] = np.ascontiguousarray(Rm.T.astype(np.float32))                    # (128, 256)
    wo1bt = np.zeros((P, 2, C), np.float32)
    for i in range(2):
        wo1bt[:, i, :] = Wo1b[:, i*P:(i+1)*P].T
    d["wo1bt"] = wo1bt.astype(ml_dtypes.bfloat16)
    d["bxo"] = np.ascontiguousarray(bias_xo.reshape(2, P).T.astype(np.float32))  # (128, 2)
    return d


def _prep_core_inputs(inp, shared):
    """Per-core (batch b, half h) sliced + padded activations."""
    x = np.asarray(inp["x"], np.float32).reshape(B, 2, P, H, W)
    xc = np.asarray(inp["x_conv"], np.float32).reshape(B, 2, P, H, W)
    in_maps = []
    for core in range(8):
        b, h = divmod(core, 2)
        r0 = 32 * h
        # window: conv out rows [r0, r0+33) -> input rows [r0-1, r0+34), cols [-1, 65)
        xwin = np.zeros((2, P, WIN_IN, WPAD), np.float32)
        lo, hi = r0 - 1, r0 + WIN_IN - 1
        slo, shi = max(lo, 0), min(hi, H)
        xwin[:, :, slo - lo:slo - lo + (shi - slo), 1:W + 1] = x[b, :, :, slo:shi, :]
        # strip: the PARTNER half's dup region, conv out rows [q0+32-D, q0+32)
        q0 = 32 * (1 - h)
        xstrip = np.zeros((2, P, STRIP_IN, WPAD), np.float32)
        lo, hi = q0 + 32 - D - 1, q0 + 32 + 1
        slo, shi = max(lo, 0), min(hi, H)
        xstrip[:, :, slo - lo:slo - lo + (shi - slo), 1:W + 1] = x[b, :, :, slo:shi, :]
        # x_conv rows [r0, r0+33), zero-padded past the image
        xconv = np.zeros((2, P, WIN_OUT, W), np.float32)
        shi = min(r0 + WIN_OUT, H)
        xconv[:, :, :shi - r0, :] = xc[b, :, :, r0:shi, :]
        m = dict(shared)
        m["xwin"] = xwin.astype(ml_dtypes.bfloat16)
        m["xstrip"] = xstrip.astype(ml_dtypes.bfloat16)
        m["xconv"] = xconv.reshape(2, P, WIN_OUT * W).astype(ml_dtypes.bfloat16)
        lastmask = np.full((P, 1), 1.0 if h == 0 else 0.0, np.float32)
        m["smalls"] = np.concatenate(
            [shared["b1"], shared["trb"], shared["bk"], shared["bxo"], lastmask,
             shared["qvbias"]], axis=1).astype(np.float32)
        m["zcol"] = np.zeros((P, WIN_OUT), ml_dtypes.bfloat16)
        for k in ("b1", "trb", "bk", "bxo", "qvbias"):
            m.pop(k, None)
        in_maps.append(m)
    return in_maps


def _build_program():
    nc = bacc.Bacc(trn_type="TRN2", num_devices=8)

    # ---- DRAM I/O ----
    t_xwin = nc.dram_tensor("xwin", [2, P, WIN_IN, WPAD], BF16, kind="ExternalInput")
    t_xstrip = nc.dram_tensor("xstrip", [2, P, STRIP_IN, WPAD], BF16, kind="ExternalInput")
    t_xconv = nc.dram_tensor("xconv", [2, P, WIN_OUT * W], BF16, kind="ExternalInput")
    t_w1t = nc.dram_tensor("w1t", [P, 2, 18, P], BF16, kind="ExternalInput")
    t_trt = nc.dram_tensor("trt", [P, 2, 18, P], BF16, kind="ExternalInput")
    t_smalls = nc.dram_tensor("smalls", [P, 264], F32, kind="ExternalInput")
    t_qvt = nc.dram_tensor("qvt", [P, 2, 2 * CH], BF16, kind="ExternalInput")
    t_wkt = nc.dram_tensor("wkt", [P, 2, CH], BF16, kind="ExternalInput")
    t_rt = nc.dram_tensor("rt", [P, 2 * CH], F32R, kind="ExternalInput")
    t_wo1bt = nc.dram_tensor("wo1bt", [P, 2, C], BF16, kind="ExternalInput")
    t_zcol = nc.dram_tensor("zcol", [P, WIN_OUT], BF16, kind="ExternalInput")
    # out[o, p, a, r, col] = output row 2a+r (within the core's 64-row half)
    t_out = nc.dram_tensor("out", [2, P, 32, 2, 2 * W], F32, kind="ExternalOutput")

    with tile.TileContext(nc) as tc:
        with (
            tc.tile_pool(name="persist", bufs=1) as pp,
            tc.tile_pool(name="qv", bufs=4) as qvp,
            tc.tile_pool(name="line", bufs=4) as linep,
        ):
            # ---- critical-path loads first, spread over queues ----
            s_w1t = pp.tile([P, 2, 18, P], BF16, tag="w1t", name="s_w1t")
            s_smalls = pp.tile([P, 264], F32, tag="smalls", name="s_smalls")
            s_b1 = s_smalls[:, 0:2]
            s_trb = s_smalls[:, 2:4]
            s_bk = s_smalls[:, 4:5]
            s_bxo = s_smalls[:, 5:7]
            s_lastmask = s_smalls[:, 7:8]
            s_xwin = [pp.tile([P, WIN_IN, WPAD], BF16, tag=f"xwin{i}", name=f"s_xwin{i}") for i in range(2)]
            s_xstrip = [pp.tile([P, STRIP_IN, WPAD], BF16, tag=f"xstrip{i}", name=f"s_xstrip{i}") for i in range(2)]
            # startup loads, fine-grained round-robin over the 3 DMA queues:
            # first-matmul set (first taps of o=0 + xwin rows 0..8) lands first,
            # then the remaining o=0/o=1 weight tap-pairs spread evenly.
            for g in range(3):
                nc.sync.dma_start(s_w1t[:, 0, 6 * g:6 * g + 6], t_w1t[:, 0, 6 * g:6 * g + 6])
            nc.gpsimd.dma_start(s_xwin[0][:, 0:9], t_xwin[0, :, 0:9])
            nc.scalar.dma_start(s_xwin[1][:, 0:9], t_xwin[1, :, 0:9])
            nc.sync.dma_start(s_smalls[:], t_smalls[:])
            for g in range(3):
                nc.sync.dma_start(s_w1t[:, 1, 6 * g:6 * g + 6], t_w1t[:, 1, 6 * g:6 * g + 6])
            nc.gpsimd.dma_start(s_xwin[0][:, 9:18], t_xwin[0, :, 9:18])
            nc.scalar.dma_start(s_xwin[1][:, 9:18], t_xwin[1, :, 9:18])
            nc.gpsimd.dma_start(s_xwin[0][:, 18:26], t_xwin[0, :, 18:26])
            nc.scalar.dma_start(s_xwin[1][:, 18:26], t_xwin[1, :, 18:26])
            nc.gpsimd.dma_start(s_xwin[0][:, 26:35], t_xwin[0, :, 26:35])
            nc.gpsimd.dma_start(s_xwin[1][:, 26:35], t_xwin[1, :, 26:35])
            s_qvt = pp.tile([P, 2, 2 * CH], BF16, tag="qvt", name="s_qvt")
            nc.sync.dma_start(s_qvt[:], t_qvt[:])
            s_qvbias2 = pp.tile([P, 512], F32, tag="qvbias", name="s_qvbias2")
            for j in range(2):
                nc.sync.dma_start(s_qvbias2[:, j * 256:(j + 1) * 256], t_smalls[:, 8:264])
            s_zcol = pp.tile([P, WIN_OUT], BF16, tag="zcol", name="s_zcol")
            nc.sync.dma_start(s_zcol[:], t_zcol[:])
            if D:
                for i in range(2):
                    nc.gpsimd.dma_start(s_xstrip[i][:], t_xstrip[i])
            s_wkt = pp.tile([P, 2, CH], BF16, tag="wkt", name="s_wkt")
            nc.sync.dma_start(s_wkt[:], t_wkt[:])
            s_rt = pp.tile([P, 2 * CH], F32R, tag="rt", name="s_rt")
            nc.sync.dma_start(s_rt[:], t_rt[:])

            s_xbwin = [pp.tile([P, WIN_OUT * W], BF16, tag=f"xbwin{o}", name=f"s_xbwin{o}") for o in range(2)]
            s_xbstrip = [pp.tile([P, max(D, 1) * W], BF16, tag=f"xbstrip{o}", name=f"s_xbstrip{o}") for o in range(2)]
            s_k = pp.tile([P, WIN_OUT * W], BF16, tag="k", name="s_k")
            s_A = pp.tile([P, CH], F32R, tag="A", name="s_A")
            s_QT = pp.tile([P, 2 * CH], BF16, tag="QT", name="s_QT")
            s_xop = [pp.tile([P, WIN_OUT, W], F32, tag=f"xop{o}", name=f"s_xop{o}") for o in range(2)]
            s_xopad = [pp.tile([P, WIN_OUT, WPAD - 1], BF16, tag=f"xopad{o}", name=f"s_xopad{o}") for o in range(2)]
            s_Asend = pp.tile([P, CH], F32R, tag="Asend", name="s_Asend")
            s_Arecv = pp.tile([P, CH], F32R, tag="Arecv", name="s_Arecv")

            def conv1(src, dst, chunks):
                """src: [2][P, rows, WPAD] padded input; dst: [2][P, out_rows*64]."""
                for o in range(2):
                    for (a0, nr) in chunks:
                        ncols = nr * W
                        ps = psMM.tile([P, 512], F32, tag="mm", name="ps_mm")
                        n_mm = 18
                        mi = 0
                        for t in range(9):
                            ky, kx = divmod(t, 3)
                            for i in range(2):
                                rhs = src[i][:, a0 + ky:a0 + ky + nr, kx:kx + W]
                                nc.tensor.matmul(
                                    ps[:, :ncols],
                                    s_w1t[:, o, t * 2 + i, :],
                                    rhs,
                                    start=(mi == 0),
                                    stop=(mi == n_mm - 1),
                                )
                                mi += 1
                        nc.any.tensor_scalar_add(
                            dst[o][:, a0 * W:a0 * W + ncols], ps[:, :ncols],
                            s_b1[:, o:o + 1],
                        )

            def strip_conv():
                for o in range(2):
                    for (a0, nr) in STRIP_CHUNKS:
                        ncols = nr * W
                        ps = psMM.tile([P, 512], F32, tag="mm", name="ps_mm")
                        mi = 0
                        for t in range(9):
                            ky, kx = divmod(t, 3)
                            for i in range(2):
                                rhs = s_xstrip[i][:, a0 + ky:a0 + ky + nr, kx:kx + W]
                                nc.tensor.matmul(
                                    ps[:, :ncols],
                                    s_w1t[:, o, t * 2 + i, :],
                                    rhs,
                                    start=(mi == 0),
                                    stop=(mi == 17),
                                )
                                mi += 1
                        nc.any.tensor_scalar_add(
                            s_xbstrip[o][:, a0 * W:a0 * W + ncols], ps[:, :ncols],
                            s_b1[:, o:o + 1],
                        )

            def gram_chunks(src_pair, col_list, ps_acc, first, last):
                """qv projection + widened Gram accumulation for 128-px chunks.
                Chunks are processed in pairs sharing one PSUM tile so the DVE
                bias-evacuation runs once per pair (512 cols)."""
                pairs = [col_list[i:i + 2] for i in range(0, len(col_list), 2)]
                for pi, pair in enumerate(pairs):
                    ps_qv = psQV.tile([P, 512], F32, tag="qv", name="ps_qv")
                    for j, col in enumerate(pair):
                        for i in range(2):
                            nc.tensor.matmul(
                                ps_qv[:, j * 256:(j + 1) * 256],
                                src_pair[i][:, col:col + P],
                                s_qvt[:, i, :],
                                start=(i == 0),
                                stop=(i == 1),
                            )
                    qv = qvp.tile([P, 512], BF16, tag="qv", name="qv_sb")
                    nw = len(pair) * 256
                    nc.vector.tensor_tensor(qv[:, :nw], ps_qv[:, :nw], s_qvbias2[:, :nw], mybir.AluOpType.add)
                    for j in range(len(pair)):
                        nc.tensor.matmul(
                            ps_acc[:],
                            qv[:, j * 256 + CH:j * 256 + 2 * CH],
                            qv[:, j * 256:(j + 1) * 256],
                            start=(first and pi == 0 and j == 0),
                            stop=(last and pi == len(pairs) - 1 and j == len(pair) - 1),
                            skip_group_check=True,
                        )

            with (
                tc.tile_pool(name="psMM", bufs=4, space="PSUM") as psMM,
                tc.tile_pool(name="psQV", bufs=2, space="PSUM") as psQV,
                tc.tile_pool(name="psA", bufs=1, space="PSUM") as psA,
            ):
                # PE p-state warm-up on a locally-memset dummy (vector memset
                # lands ~+0.3us, long before the first weights at ~+10us), so
                # the PE ramps to full clock DURING the DMA shadow instead of
                # issuing its first real matmuls at the 2-3x cold rate. 12
                # back-to-back 512-col matmuls ~= 6-7us of continuous busy,
                # ending just as w1t lands. Results unread.
                s_warm = pp.tile([P, 512], BF16, tag="warm", name="s_warm")
                nc.vector.memset(s_warm[:], 0.0)
                for _wi in range(12):
                    psw = psMM.tile([P, 512], F32, tag="mm", name="ps_mm")
                    nc.tensor.matmul(
                        psw[:], s_warm[:, 0:128], s_warm[:],
                        start=True, stop=True,
                    )
                # ---- pass-1: conv of S_cc rows + their Gram, then launch AR ----
                conv1(s_xwin, s_xbwin, CC_CHUNKS)
                ps_Acc = psA.tile([P, 2 * CH], F32, tag="Acc", name="ps_Acc")
                gram_chunks(s_xbwin, [128 * i for i in range(N_G1)], ps_Acc, True, True)
                nc.vector.tensor_copy(s_Asend[:], ps_Acc[:, :CH])
                # pairwise exchange of the S_cc Gram partial via SWDGE remote
                # DMA (SBUF->peer SBUF, ~us) instead of the ncfw collective
                # (~30-45us trigger-to-usable). Relative dest (drid=0, dtpb=1):
                # Q7 XORs with own tpb, and the rank pairing (2i, 2i+1) maps to
                # phys-tpb pairs differing in bit 0 under every trn2 layout.
                # The kernel-entry barrier (prelude AllGather on the CC stream,
                # concurrent with the ~45us of pre-exchange compute) guarantees
                # the peer has cleared its sems before our sem update arrives.
                # The barrier wait + recv-sem wait are invisible to the Tile
                # scheduling sim (incremented by compile-time prelude / remote
                # peer) and would deadlock it -- they are spliced in after
                # scheduling, before the captured trigger / QT matmul below.
                recv_sem = nc.alloc_semaphore("agram_recv")
                send_sem = nc.alloc_semaphore("agram_send_done")
                # queue 1 keeps the untriggered prep off the SWDGE ring that
                # regular gpsimd DMAs auto-trigger through
                nc.gpsimd.remote_dma_broadcast(
                    s_Arecv[:],
                    s_Asend[:],
                    remote_sem=recv_sem,
                    local_sem=send_sem,
                    rdests=[(0, 1), None, None, None, None, None, None, None],
                )
                ins_trigger = nc.gpsimd.trigger_dma(count=None).ins

                # ---- cover phase (exchange in flight) ----
                conv1(s_xwin, s_xbwin, TAIL_CHUNKS)
                if D:
                    strip_conv()
                    ps_Adup = psA.tile([P, 2 * CH], F32, tag="Adup", name="ps_Adup")
                    gram_chunks(s_xbwin, G2_OWN_COLS, ps_Adup, True, False)
                    gram_chunks(s_xbstrip, G2_STRIP_COLS, ps_Adup, False, True)

                # k = Wk @ xb_win + Bk
                for (a0, nr) in K_CHUNKS:
                    c0, ncols = a0 * W, nr * W
                    ps = psMM.tile([P, 512], F32, tag="mm", name="ps_mm")
                    for i in range(2):
                        nc.tensor.matmul(
                            ps[:, :ncols],
                            s_wkt[:, i, :],
                            s_xbwin[i][:, c0:c0 + ncols],
                            start=(i == 0),
                            stop=(i == 1),
                        )
                    nc.any.tensor_scalar_add(s_k[:, c0:c0 + ncols], ps[:, :ncols], s_bk[:])

                # x_conv half of xo -> s_xop (bias included, no mask yet)
                s_xconv = [pp.tile([P, WIN_OUT * W], BF16, tag=f"xconv{i}", name=f"s_xconv{i}") for i in range(2)]
                for i in range(2):
                    nc.sync.dma_start(s_xconv[i][:], t_xconv[i])
                s_wo1bt = pp.tile([P, 2, C], BF16, tag="wo1bt", name="s_wo1bt")
                nc.scalar.dma_start(s_wo1bt[:], t_wo1bt[:])
                s_trt = pp.tile([P, 2, 18, P], BF16, tag="trt", name="s_trt")
                for o in range(2):
                    nc.sync.dma_start(s_trt[:, o], t_trt[:, o])
                for o in range(2):
                    # right pad column must be zero (memset can't encode f32r);
                    # strided DMA would be 33x128 4-byte descriptors - use DVE
                    nc.vector.tensor_copy(s_xopad[o][:, :, W:W + 1], s_zcol[:, :, None])

                for o in range(2):
                    for (a0, nr) in XOP_CHUNKS:
                        c0, ncols = a0 * W, nr * W
                        ps = psMM.tile([P, 512], F32, tag="mm", name="ps_mm")
                        for i in range(2):
                            nc.tensor.matmul(
                                ps[:, :ncols],
                                s_wo1bt[:, i, o * CH:(o + 1) * CH],
                                s_xconv[i][:, c0:c0 + ncols],
                                start=(i == 0), stop=(i == 1),
                            )
                        nc.any.tensor_scalar_add(
                            s_xop[o][:, a0:a0 + nr, :],
                            ps[:, :ncols].rearrange("p (a w) -> p a w", w=W),
                            s_bxo[:, o:o + 1],
                        )

                # ---- QT = (Rm @ (A_dup? + A_own + A_peer))^T ----
                ps_qt = psQV.tile([P, 2 * CH], F32, tag="qv", name="ps_qt")
                if D:
                    # evacuate the local dup Gram during the exchange flight
                    nc.any.tensor_copy(s_A[:], ps_Adup[:, :CH])
                    nc.tensor.matmul(ps_qt[:], s_A[:], s_rt[:], start=True, stop=False)
                    nc.tensor.matmul(ps_qt[:], s_Asend[:], s_rt[:], start=False, stop=False)
                else:
                    nc.tensor.matmul(ps_qt[:], s_Asend[:], s_rt[:], start=True, stop=False)
                ins_qt_recv = nc.tensor.matmul(
                    ps_qt[:], s_Arecv[:], s_rt[:], start=False, stop=True
                ).ins
                nc.any.tensor_copy(s_QT[:], ps_qt[:])

            # ---- phase 2: attn half of xo + convT with a deeper PSUM pool ----
            with tc.tile_pool(name="psMM2", bufs=6, space="PSUM") as psMM2:
                for o in range(2):
                    for (a0, nr) in XO_CHUNKS:
                        c0, ncols = a0 * W, nr * W
                        ps = psMM2.tile([P, 512], F32, tag="mm2", name="ps_mm2")
                        nc.tensor.matmul(
                            ps[:, :ncols],
                            s_QT[:, o * CH:(o + 1) * CH],
                            s_k[:, c0:c0 + ncols],
                            start=True, stop=True,
                        )
                        dst = s_xopad[o][:, a0:a0 + nr, 0:W]
                        src2 = ps[:, :ncols].rearrange("p (a w) -> p a w", w=W)
                        nc.any.tensor_tensor(dst, src2, s_xop[o][:, a0:a0 + nr, :], mybir.AluOpType.add)
                        if a0 + nr == WIN_OUT:
                            # halo row (row 32) is zero on the bottom-half core
                            hd = s_xopad[o][:, WIN_OUT - 1:WIN_OUT, 0:W]
                            nc.any.tensor_scalar(
                                hd, hd, s_lastmask[:], None,
                                op0=mybir.AluOpType.mult,
                            )

                # ---- convT: 4 parity grids over local a in [0, 32) ----
                for r in range(2):
                    for a0 in (0, 8, 16, 24):
                        for o in range(2):
                            line = linep.tile([P, 8, 2 * W], F32, tag="line", name="line")
                            for s in range(2):
                                taps = CT_TAPS[(r, s)]
                                ps = psMM2.tile([P, 512], F32, tag="mm2", name="ps_mm2")
                                n_mm = len(taps) * 2
                                mi = 0
                                for (ky, kx, da, db) in taps:
                                    t = ky * 3 + kx
                                    for i in range(2):
                                        rhs = s_xopad[i][:, a0 + da:a0 + da + 8, db:db + W]
                                        nc.tensor.matmul(
                                            ps[:],
                                            s_trt[:, o, t * 2 + i, :],
                                            rhs,
                                            start=(mi == 0),
                                            stop=(mi == n_mm - 1),
                                        )
                                        mi += 1
                                nc.any.tensor_scalar_add(
                                    line[:, :, s::2],
                                    ps.rearrange("p (a w) -> p a w", w=W),
                                    s_trb[:, o:o + 1],
                                )
                            e0, e1 = ((nc.sync, nc.gpsimd), (nc.gpsimd, nc.scalar),
                                      (nc.scalar, nc.sync))[(r * 8 + a0 // 8 * 2 + o) % 3]
                            e0.dma_start(t_out[o, :, a0:a0 + 4, r, :], line[:, 0:4])
                            e1.dma_start(t_out[o, :, a0 + 4:a0 + 8, r, :], line[:, 4:8])

    # Splice in the two externally-incremented sem waits the Tile sim could
    # not model: (a) gate the SWDGE trigger on the kernel-entry barrier so the
    # peer has cleared its sems before our remote write+sem-inc arrives;
    # (b) gate the QT matmul that reads s_Arecv on the peer's data landing
    # (remote_sem += 2, one per DMA lane of slot 0).
    nc._bir_kernel_barrier_sem_replica_groups.extend(
        set(g) for g in [[0, 1], [2, 3], [4, 5], [6, 7]]
    )
    w_bar = nc.gpsimd.wait_ge(
        nc._bir_kernel_barrier_sem, nc.bir_kernel_barrier_sem_inc
    ).ins
    w_recv = nc.tensor.wait_ge(recv_sem, 2).ins

    def _move_before(wait_ins, target_ins):
        blocks = nc.main_func.blocks
        for b in blocks:
            if wait_ins in b.instructions:
                b.instructions.remove(wait_ins)
                break
        for b in blocks:
            if target_ins in b.instructions:
                idx = b.instructions.index(target_ins)
                b.instructions.insert(idx, wait_ins)
                return
        raise RuntimeError("target instruction not found in any block")

    _move_before(w_bar, ins_trigger)
    _move_before(w_recv, ins_qt_recv)

    nc.compile()
    return nc


def _ensure_ntff_hook():
    """antenv.axon_hooks is absent in this image; recreate it + install the
    ctypes NTFF hook so run_bass_kernel_spmd(trace=True) can profile."""
    try:
        from antenv import axon_hooks  # noqa: F401
        return
    except ImportError:
        pass
    try:
        import types
        import antenv
        mod = types.ModuleType("antenv.axon_hooks")
        _hook = [None]
        mod.set_axon_ntff_profile_hook = lambda h: _hook.__setitem__(0, h)
        mod.get_axon_ntff_profile_hook = lambda: _hook[0]
        sys.modules["antenv.axon_hooks"] = mod
        antenv.axon_hooks = mod
        from trn_agent_boot.trn_boot import _ntff_profile_via_ctypes
        mod.set_axon_ntff_profile_hook(
            _ntff_profile_via_ctypes("/opt/axon/libaxon_pjrt.so")
        )
    except Exception:
        pass


def kernel(**inputs):
    global LAST_EXEC_TIME_NS, LAST_PROFILE
    if "nc" not in _CACHE:
        _CACHE["nc"] = _build_program()
    nc = _CACHE["nc"]
    shared = _prep_weights(inputs)
    in_maps = _prep_core_inputs(inputs, shared)
    trace = os.environ.get("KERNEL_PROFILE", "") in ("1", "true")
    if trace:
        _ensure_ntff_hook()
    res = run_bass_kernel_spmd(nc, in_maps, core_ids=list(range(8)), trace=trace)
    LAST_EXEC_TIME_NS = getattr(res, "exec_time_ns", None)
    LAST_PROFILE = getattr(res, "profile_json", None)
    out = np.zeros((B, C, 2 * H, 2 * W), np.float32)
    for core in range(8):
        b, h = divmod(core, 2)
        o = res.results[core]["out"]  # (2, 128, 32, 2, 128)
        out[b, :, 64 * h:64 * (h + 1), :] = o.reshape(C, 64, 2 * W)
    return out


if __name__ == "__main__":
    print("smoke build only")
    _build_program()
    print("build ok")



# revision 75
# speedup vs baseline: 1.0742x; 1.0742x over previous
"""Trainium2 Bass kernel for nn_Attention_decoder (conv + linear-attention + convT block).

Math refactoring (validated vs reference):
  - All BatchNorms folded into weights/biases (eval mode, affine).
  - No softmax => the two N^2 einsums collapse by associativity:
      m1 = vf^T @ (qf @ k) = (vf^T qf) @ k = A @ k,  A is 128x128.
  - out-proj and out1 folded:  xo = (Wo1a@out_w/16 @ A) @ k + Wo1b @ x_conv + bias.
  - ConvTranspose2d(k=3,s=2,p=1,op=1) decomposed into 4 parity sub-convs.

Sharding: 8 cores = 4 batches x 2 row-halves. The attention Gram matrix
A = vf^T qf is spatially global (all 64 rows); each core owns 32 rows.
Schedule (cross-core exchange):
  - Gram split: S_cc = each core's first 32-D window rows (exchanged with
    the pair partner) + S_dup = the last D rows of BOTH halves, computed
    locally by both cores (own tail from the window conv + a D-row strip
    of the partner's tail conv'd redundantly).
  - The S_cc partial is exchanged pairwise via SWDGE remote_dma_broadcast
    (SBUF -> peer SBUF, relative dest drid=0/dtpb=1, ~2us) instead of a
    ncfw collective_compute AllReduce (~35-45us trigger-to-usable). The
    send trigger is gated on the kernel-entry barrier (a 1-byte prelude
    AllGather on the CC stream, done ~+55us, concurrent with compute) so
    the peer has cleared its sems before the remote sem-inc arrives; both
    that wait and the recv-sem wait are spliced in AFTER Tile scheduling
    (the Tile sim cannot see remote/prelude increments and would deadlock).
  - After the exchange: A = own S_cc + peer S_cc + local dup Gram -> QT,
    attn half of xo, and the transposed conv.
Dtypes: conv1/convT weights+inputs and the whole mid-section (xb, qv, k,
xconv, QT) are bf16 (1 cycle/row matmuls, 4x cheaper LDWEIGHTS than f32r,
half the DMA bytes); PSUM accumulation stays fp32; the exchange payload
and QT accumulation stay f32r. rel_err ~5e-3 vs the 2e-2 gate.
NOTE: D=4/D=6 and some DMA-queue replans hang the NEFF (schedule-sensitive
race somewhere around the exchange) -- D=8 with this exact load plan is
stable across runs. Perturb with care and re-verify on HW.
"""

import os
import sys

for _p in ("/opt/trn_rl_repo", "/root/.axon_site/_ro/trn_rl_repo"):
    if os.path.isdir(_p) and _p not in sys.path:
        sys.path.insert(0, _p)

import ml_dtypes
import numpy as np

import concourse.bass as bass
import concourse.mybir as mybir
import concourse.tile as tile
from concourse import bacc
from concourse.bass_utils import run_bass_kernel_spmd

EPS = 1e-5
B, C, H, W = 4, 256, 64, 64
CH = 128  # attention head dim
P = 128   # partitions
F32 = mybir.dt.float32
F32R = mybir.dt.float32r
BF16 = mybir.dt.bfloat16

D = 2          # dup strip rows per half (exchange-latency cover)
WIN_IN = 35    # input rows for the window conv (33 out rows)
WIN_OUT = 33   # window out rows: 32 half rows + 1 halo
STRIP_IN = D + 2
WPAD = 66      # horizontally padded width


def _rowchunks(r0, r1, maxr=7):
    out = []
    while r0 < r1:
        n = min(maxr, r1 - r0)
        out.append((r0, n))
        r0 += n
    return out


# conv chunking (rows): pass-1 = S_cc rows [0, 32-D); tail = [32-D, 33)
# NB: D in {0, 4, 6} hangs the NEFF on HW regardless of tail or strip
# chunking (both the 2-tail-chunk and 2-strip-chunk theories were tested
# and falsified). Only D=8 and D=12 are known-stable; D=8 is faster. The
# D=0 path below would save ~8.5us if the hang is ever root-caused.
CC_CHUNKS = _rowchunks(0, 32 - D)
TAIL_CHUNKS = _rowchunks(32 - D, 33)
STRIP_CHUNKS = _rowchunks(0, D)
N_G1 = (32 - D) * 64 // 128          # Gram chunks before the exchange
G2_OWN_COLS = [(32 - D) * 64 + 128 * i for i in range(D // 2)]
G2_STRIP_COLS = [128 * i for i in range(D // 2)]
K_CHUNKS = [(0, 7), (7, 7), (14, 7), (21, 6), (27, 6)]
XOP_CHUNKS = [(0, 7), (7, 7), (14, 7), (21, 6), (27, 6)]   # x_conv partials
XO_CHUNKS = [(0, 7), (7, 7), (14, 7), (21, 6), (27, 6)]  # attn+merge

# convT parity grid taps: (r, s) -> [(ky, kx, da, db), ...]
CT_TAPS = {
    (0, 0): [(1, 1, 0, 0)],
    (0, 1): [(1, 0, 0, 1), (1, 2, 0, 0)],
    (1, 0): [(0, 1, 1, 0), (2, 1, 0, 0)],
    (1, 1): [(0, 0, 1, 1), (0, 2, 1, 0), (2, 0, 0, 1), (2, 2, 0, 0)],
}

LAST_EXEC_TIME_NS = None
LAST_PROFILE = None

_CACHE = {}


def _fold_bn(g, b, m, v):
    s = g / np.sqrt(v + EPS)
    return s.astype(np.float64), (b - m * s).astype(np.float64)


def _prep_weights(inp):
    """Host-side BN folding + layout prep. Returns dict of shared per-core arrays."""
    f8 = lambda a: np.asarray(a, np.float64)
    s1, t1 = _fold_bn(f8(inp["bn1_g"]), f8(inp["bn1_b"]), f8(inp["bn1_m"]), f8(inp["bn1_v"]))
    W1 = f8(inp["conv_w"]) * s1[:, None, None, None]          # (co, ci, ky, kx)
    B1 = s1 * f8(inp["conv_b"]) + t1                          # (256,)
    sq, tq = _fold_bn(f8(inp["qbn_g"]), f8(inp["qbn_b"]), f8(inp["qbn_m"]), f8(inp["qbn_v"]))
    Wq = f8(inp["q_w"]) * sq[:, None]
    Bq = sq * f8(inp["q_b"]) + tq
    sk, tk = _fold_bn(f8(inp["kbn_g"]), f8(inp["kbn_b"]), f8(inp["kbn_m"]), f8(inp["kbn_v"]))
    Wk = f8(inp["k_w"]) * sk[:, None]
    Bk = sk * f8(inp["k_b"]) + tk
    Wv = f8(inp["v_w"])
    Bv = f8(inp["v_b"])
    so, to = _fold_bn(f8(inp["obn_g"]), f8(inp["obn_b"]), f8(inp["obn_m"]), f8(inp["obn_v"]))
    Wo1 = f8(inp["out1_w"]) * so[:, None]                     # (256, 512)
    Bo1 = so * f8(inp["out1_b"]) + to
    Wo1a, Wo1b = Wo1[:, :C], Wo1[:, C:]
    Rm = Wo1a @ f8(inp["out_w"]) / 16.0                       # (256, 128)
    bias_xo = Bo1 + Wo1a @ f8(inp["out_b"])                   # (256,)
    trw = f8(inp["tr_w"])                                     # (ci, co, ky, kx)
    trb = f8(inp["tr_b"])

    d = {}
    # conv1 lhsT: w1t[a, o, t*2 + i, b] = W1[o*128+b, i*128+a, ky, kx]
    w1t = np.zeros((P, 2, 18, P), np.float32)
    trt = np.zeros((P, 2, 18, P), np.float32)
    for t in range(9):
        ky, kx = divmod(t, 3)
        for i in range(2):
            for o in range(2):
                idx = t * 2 + i
                w1t[:, o, idx, :] = W1[o*P:(o+1)*P, i*P:(i+1)*P, ky, kx].T
                trt[:, o, idx, :] = trw[i*P:(i+1)*P, o*P:(o+1)*P, ky, kx]
    d["w1t"] = w1t.astype(ml_dtypes.bfloat16)
    d["trt"] = trt.astype(ml_dtypes.bfloat16)
    d["b1"] = np.ascontiguousarray(B1.reshape(2, P).T.astype(np.float32))      # (128, 2)
    d["trb"] = np.ascontiguousarray(trb.reshape(2, P).T.astype(np.float32))    # (128, 2)
    # qv moving weights: qvt[a, i, 0:128]=Wq[c, i*128+a]; [...,128:256]=Wv
    qvt = np.zeros((P, 2, 2 * CH), np.float32)
    for i in range(2):
        qvt[:, i, :CH] = Wq[:, i*P:(i+1)*P].T
        qvt[:, i, CH:] = Wv[:, i*P:(i+1)*P].T
    d["qvt"] = qvt.astype(ml_dtypes.bfloat16)
    d["qvbias"] = np.broadcast_to(
        np.concatenate([Bq, Bv]).astype(np.float32)[None, :], (P, 2 * CH)
    ).copy()
    wkt = np.zeros((P, 2, CH), np.float32)
    for i in range(2):
        wkt[:, i, :] = Wk[:, i*P:(i+1)*P].T
    d["wkt"] = wkt.astype(ml_dtypes.bfloat16)
    d["bk"] = Bk.astype(np.float32).reshape(P, 1)  # packed into smalls below
    d["rt"] = np.ascontiguousarray(Rm.T.astype(np.float32))                    # (128, 256)
    wo1bt = np.zeros((P, 2, C), np.float32)
    for i in range(2):
        wo1bt[:, i, :] = Wo1b[:, i*P:(i+1)*P].T
    d["wo1bt"] = wo1bt.astype(ml_dtypes.bfloat16)
    d["bxo"] = np.ascontiguousarray(bias_xo.reshape(2, P).T.astype(np.float32))  # (128, 2)
    return d


def _prep_core_inputs(inp, shared):
    """Per-core (batch b, half h) sliced + padded activations."""
    x = np.asarray(inp["x"], np.float32).reshape(B, 2, P, H, W)
    xc = np.asarray(inp["x_conv"], np.float32).reshape(B, 2, P, H, W)
    in_maps = []
    for core in range(8):
        b, h = divmod(core, 2)
        r0 = 32 * h
        # window: conv out rows [r0, r0+33) -> input rows [r0-1, r0+34), cols [-1, 65)
        xwin = np.zeros((2, P, WIN_IN, WPAD), np.float32)
        lo, hi = r0 - 1, r0 + WIN_IN - 1
        slo, shi = max(lo, 0), min(hi, H)
        xwin[:, :, slo - lo:slo - lo + (shi - slo), 1:W + 1] = x[b, :, :, slo:shi, :]
        # strip: the PARTNER half's dup region, conv out rows [q0+32-D, q0+32)
        q0 = 32 * (1 - h)
        xstrip = np.zeros((2, P, STRIP_IN, WPAD), np.float32)
        lo, hi = q0 + 32 - D - 1, q0 + 32 + 1
        slo, shi = max(lo, 0), min(hi, H)
        xstrip[:, :, slo - lo:slo - lo + (shi - slo), 1:W + 1] = x[b, :, :, slo:shi, :]
        # x_conv rows [r0, r0+33), zero-padded past the image
        xconv = np.zeros((2, P, WIN_OUT, W), np.float32)
        shi = min(r0 + WIN_OUT, H)
        xconv[:, :, :shi - r0, :] = xc[b, :, :, r0:shi, :]
        m = dict(shared)
        m["xwin"] = xwin.astype(ml_dtypes.bfloat16)
        m["xstrip"] = xstrip.astype(ml_dtypes.bfloat16)
        m["xconv"] = xconv.reshape(2, P, WIN_OUT * W).astype(ml_dtypes.bfloat16)
        lastmask = np.full((P, 1), 1.0 if h == 0 else 0.0, np.float32)
        m["smalls"] = np.concatenate(
            [shared["b1"], shared["trb"], shared["bk"], shared["bxo"], lastmask,
             shared["qvbias"]], axis=1).astype(np.float32)
        m["zcol"] = np.zeros((P, WIN_OUT), ml_dtypes.bfloat16)
        for k in ("b1", "trb", "bk", "bxo", "qvbias"):
            m.pop(k, None)
        in_maps.append(m)
    return in_maps


def _build_program():
    nc = bacc.Bacc(trn_type="TRN2", num_devices=8, num_swdge_queues=2)

    # ---- DRAM I/O ----
    t_xwin = nc.dram_tensor("xwin", [2, P, WIN_IN, WPAD], BF16, kind="ExternalInput")
    t_xstrip = nc.dram_tensor("xstrip", [2, P, STRIP_IN, WPAD], BF16, kind="ExternalInput")
    t_xconv = nc.dram_tensor("xconv", [2, P, WIN_OUT * W], BF16, kind="ExternalInput")
    t_w1t = nc.dram_tensor("w1t", [P, 2, 18, P], BF16, kind="ExternalInput")
    t_trt = nc.dram_tensor("trt", [P, 2, 18, P], BF16, kind="ExternalInput")
    t_smalls = nc.dram_tensor("smalls", [P, 264], F32, kind="ExternalInput")
    t_qvt = nc.dram_tensor("qvt", [P, 2, 2 * CH], BF16, kind="ExternalInput")
    t_wkt = nc.dram_tensor("wkt", [P, 2, CH], BF16, kind="ExternalInput")
    t_rt = nc.dram_tensor("rt", [P, 2 * CH], F32R, kind="ExternalInput")
    t_wo1bt = nc.dram_tensor("wo1bt", [P, 2, C], BF16, kind="ExternalInput")
    t_zcol = nc.dram_tensor("zcol", [P, WIN_OUT], BF16, kind="ExternalInput")
    # out[o, p, a, r, col] = output row 2a+r (within the core's 64-row half)
    t_out = nc.dram_tensor("out", [2, P, 32, 2, 2 * W], F32, kind="ExternalOutput")

    with tile.TileContext(nc) as tc:
        with (
            tc.tile_pool(name="persist", bufs=1) as pp,
            tc.tile_pool(name="qv", bufs=4) as qvp,
            tc.tile_pool(name="line", bufs=4) as linep,
        ):
            # ---- critical-path loads first, spread over queues ----
            s_w1t = pp.tile([P, 2, 18, P], BF16, tag="w1t", name="s_w1t")
            s_smalls = pp.tile([P, 264], F32, tag="smalls", name="s_smalls")
            s_b1 = s_smalls[:, 0:2]
            s_trb = s_smalls[:, 2:4]
            s_bk = s_smalls[:, 4:5]
            s_bxo = s_smalls[:, 5:7]
            s_lastmask = s_smalls[:, 7:8]
            s_xwin = [pp.tile([P, WIN_IN, WPAD], BF16, tag=f"xwin{i}", name=f"s_xwin{i}") for i in range(2)]
            s_xstrip = [pp.tile([P, STRIP_IN, WPAD], BF16, tag=f"xstrip{i}", name=f"s_xstrip{i}") for i in range(2)]
            # startup loads, fine-grained round-robin over the 3 DMA queues:
            # first-matmul set (first taps of o=0 + xwin rows 0..8) lands first,
            # then the remaining o=0/o=1 weight tap-pairs spread evenly.
            for g in range(3):
                nc.sync.dma_start(s_w1t[:, 0, 6 * g:6 * g + 6], t_w1t[:, 0, 6 * g:6 * g + 6])
            nc.gpsimd.dma_start(s_xwin[0][:, 0:9], t_xwin[0, :, 0:9])
            nc.scalar.dma_start(s_xwin[1][:, 0:9], t_xwin[1, :, 0:9])
            nc.sync.dma_start(s_smalls[:], t_smalls[:])
            for g in range(3):
                nc.sync.dma_start(s_w1t[:, 1, 6 * g:6 * g + 6], t_w1t[:, 1, 6 * g:6 * g + 6])
            nc.gpsimd.dma_start(s_xwin[0][:, 9:18], t_xwin[0, :, 9:18])
            nc.scalar.dma_start(s_xwin[1][:, 9:18], t_xwin[1, :, 9:18])
            nc.gpsimd.dma_start(s_xwin[0][:, 18:26], t_xwin[0, :, 18:26])
            nc.scalar.dma_start(s_xwin[1][:, 18:26], t_xwin[1, :, 18:26])
            nc.gpsimd.dma_start(s_xwin[0][:, 26:35], t_xwin[0, :, 26:35])
            nc.gpsimd.dma_start(s_xwin[1][:, 26:35], t_xwin[1, :, 26:35])
            s_qvt = pp.tile([P, 2, 2 * CH], BF16, tag="qvt", name="s_qvt")
            nc.sync.dma_start(s_qvt[:], t_qvt[:])
            s_qvbias2 = pp.tile([P, 512], F32, tag="qvbias", name="s_qvbias2")
            for j in range(2):
                nc.sync.dma_start(s_qvbias2[:, j * 256:(j + 1) * 256], t_smalls[:, 8:264])
            s_zcol = pp.tile([P, WIN_OUT], BF16, tag="zcol", name="s_zcol")
            nc.sync.dma_start(s_zcol[:], t_zcol[:])
            if D:
                for i in range(2):
                    nc.gpsimd.dma_start(s_xstrip[i][:], t_xstrip[i])
            s_wkt = pp.tile([P, 2, CH], BF16, tag="wkt", name="s_wkt")
            nc.sync.dma_start(s_wkt[:], t_wkt[:])
            s_rt = pp.tile([P, 2 * CH], F32R, tag="rt", name="s_rt")
            nc.sync.dma_start(s_rt[:], t_rt[:])

            s_xbwin = [pp.tile([P, WIN_OUT * W], BF16, tag=f"xbwin{o}", name=f"s_xbwin{o}") for o in range(2)]
            s_xbstrip = [pp.tile([P, max(D, 1) * W], BF16, tag=f"xbstrip{o}", name=f"s_xbstrip{o}") for o in range(2)]
            s_k = pp.tile([P, WIN_OUT * W], BF16, tag="k", name="s_k")
            s_A = pp.tile([P, CH], F32R, tag="A", name="s_A")
            s_QT = pp.tile([P, 2 * CH], BF16, tag="QT", name="s_QT")
            s_xop = [pp.tile([P, WIN_OUT, W], F32, tag=f"xop{o}", name=f"s_xop{o}") for o in range(2)]
            s_xopad = [pp.tile([P, WIN_OUT, WPAD - 1], BF16, tag=f"xopad{o}", name=f"s_xopad{o}") for o in range(2)]
            s_Asend = pp.tile([P, CH], F32R, tag="Asend", name="s_Asend")
            s_Arecv = pp.tile([P, CH], F32R, tag="Arecv", name="s_Arecv")

            def conv1(src, dst, chunks):
                """src: [2][P, rows, WPAD] padded input; dst: [2][P, out_rows*64]."""
                for o in range(2):
                    for (a0, nr) in chunks:
                        ncols = nr * W
                        ps = psMM.tile([P, 512], F32, tag="mm", name="ps_mm")
                        n_mm = 18
                        mi = 0
                        for t in range(9):
                            ky, kx = divmod(t, 3)
                            for i in range(2):
                                rhs = src[i][:, a0 + ky:a0 + ky + nr, kx:kx + W]
                                nc.tensor.matmul(
                                    ps[:, :ncols],
                                    s_w1t[:, o, t * 2 + i, :],
                                    rhs,
                                    start=(mi == 0),
                                    stop=(mi == n_mm - 1),
                                )
                                mi += 1
                        nc.any.tensor_scalar_add(
                            dst[o][:, a0 * W:a0 * W + ncols], ps[:, :ncols],
                            s_b1[:, o:o + 1],
                        )

            def strip_conv():
                for o in range(2):
                    for (a0, nr) in STRIP_CHUNKS:
                        ncols = nr * W
                        ps = psMM.tile([P, 512], F32, tag="mm", name="ps_mm")
                        mi = 0
                        for t in range(9):
                            ky, kx = divmod(t, 3)
                            for i in range(2):
                                rhs = s_xstrip[i][:, a0 + ky:a0 + ky + nr, kx:kx + W]
                                nc.tensor.matmul(
                                    ps[:, :ncols],
                                    s_w1t[:, o, t * 2 + i, :],
                                    rhs,
                                    start=(mi == 0),
                                    stop=(mi == 17),
                                )
                                mi += 1
                        nc.any.tensor_scalar_add(
                            s_xbstrip[o][:, a0 * W:a0 * W + ncols], ps[:, :ncols],
                            s_b1[:, o:o + 1],
                        )

            def gram_chunks(src_pair, col_list, ps_acc, first, last):
                """qv projection + widened Gram accumulation for 128-px chunks.
                Chunks are processed in pairs sharing one PSUM tile so the DVE
                bias-evacuation runs once per pair (512 cols)."""
                pairs = [col_list[i:i + 2] for i in range(0, len(col_list), 2)]
                for pi, pair in enumerate(pairs):
                    ps_qv = psQV.tile([P, 512], F32, tag="qv", name="ps_qv")
                    for j, col in enumerate(pair):
                        for i in range(2):
                            nc.tensor.matmul(
                                ps_qv[:, j * 256:(j + 1) * 256],
                                src_pair[i][:, col:col + P],
                                s_qvt[:, i, :],
                                start=(i == 0),
                                stop=(i == 1),
                            )
                    qv = qvp.tile([P, 512], BF16, tag="qv", name="qv_sb")
                    nw = len(pair) * 256
                    nc.vector.tensor_tensor(qv[:, :nw], ps_qv[:, :nw], s_qvbias2[:, :nw], mybir.AluOpType.add)
                    for j in range(len(pair)):
                        nc.tensor.matmul(
                            ps_acc[:],
                            qv[:, j * 256 + CH:j * 256 + 2 * CH],
                            qv[:, j * 256:(j + 1) * 256],
                            start=(first and pi == 0 and j == 0),
                            stop=(last and pi == len(pairs) - 1 and j == len(pair) - 1),
                            skip_group_check=True,
                        )

            with (
                tc.tile_pool(name="psMM", bufs=4, space="PSUM") as psMM,
                tc.tile_pool(name="psQV", bufs=2, space="PSUM") as psQV,
                tc.tile_pool(name="psA", bufs=1, space="PSUM") as psA,
            ):
                # PE p-state warm-up on a locally-memset dummy (vector memset
                # lands ~+0.3us, long before the first weights at ~+10us), so
                # the PE ramps to full clock DURING the DMA shadow instead of
                # issuing its first real matmuls at the 2-3x cold rate. 12
                # back-to-back 512-col matmuls ~= 6-7us of continuous busy,
                # ending just as w1t lands. Results unread.
                s_warm = pp.tile([P, 512], BF16, tag="warm", name="s_warm")
                nc.vector.memset(s_warm[:], 0.0)
                for _wi in range(12):
                    psw = psMM.tile([P, 512], F32, tag="mm", name="ps_mm")
                    nc.tensor.matmul(
                        psw[:], s_warm[:, 0:128], s_warm[:],
                        start=True, stop=True,
                    )
                # ---- pass-1: conv of S_cc rows + their Gram, then launch AR ----
                conv1(s_xwin, s_xbwin, CC_CHUNKS)
                ps_Acc = psA.tile([P, 2 * CH], F32, tag="Acc", name="ps_Acc")
                gram_chunks(s_xbwin, [128 * i for i in range(N_G1)], ps_Acc, True, True)
                nc.vector.tensor_copy(s_Asend[:], ps_Acc[:, :CH])
                # pairwise exchange of the S_cc Gram partial via SWDGE remote
                # DMA (SBUF->peer SBUF, ~us) instead of the ncfw collective
                # (~30-45us trigger-to-usable). Relative dest (drid=0, dtpb=1):
                # Q7 XORs with own tpb, and the rank pairing (2i, 2i+1) maps to
                # phys-tpb pairs differing in bit 0 under every trn2 layout.
                # The kernel-entry barrier (prelude AllGather on the CC stream,
                # concurrent with the ~45us of pre-exchange compute) guarantees
                # the peer has cleared its sems before our sem update arrives.
                # The barrier wait + recv-sem wait are invisible to the Tile
                # scheduling sim (incremented by compile-time prelude / remote
                # peer) and would deadlock it -- they are spliced in after
                # scheduling, before the captured trigger / QT matmul below.
                recv_sem = nc.alloc_semaphore("agram_recv")
                send_sem = nc.alloc_semaphore("agram_send_done")
                # queue 1 keeps the untriggered prep off the SWDGE ring that
                # regular gpsimd DMAs auto-trigger through
                nc.gpsimd.remote_dma_broadcast(
                    s_Arecv[:],
                    s_Asend[:],
                    remote_sem=recv_sem,
                    local_sem=send_sem,
                    rdests=[(0, 1), None, None, None, None, None, None, None],
                    queue_num=1,
                )
                ins_trigger = nc.gpsimd.trigger_dma(count=None, queue_num=1).ins

                # ---- cover phase (exchange in flight) ----
                conv1(s_xwin, s_xbwin, TAIL_CHUNKS)
                if D:
                    strip_conv()
                    ps_Adup = psA.tile([P, 2 * CH], F32, tag="Adup", name="ps_Adup")
                    gram_chunks(s_xbwin, G2_OWN_COLS, ps_Adup, True, False)
                    gram_chunks(s_xbstrip, G2_STRIP_COLS, ps_Adup, False, True)

                # k = Wk @ xb_win + Bk
                for (a0, nr) in K_CHUNKS:
                    c0, ncols = a0 * W, nr * W
                    ps = psMM.tile([P, 512], F32, tag="mm", name="ps_mm")
                    for i in range(2):
                        nc.tensor.matmul(
                            ps[:, :ncols],
                            s_wkt[:, i, :],
                            s_xbwin[i][:, c0:c0 + ncols],
                            start=(i == 0),
                            stop=(i == 1),
                        )
                    nc.any.tensor_scalar_add(s_k[:, c0:c0 + ncols], ps[:, :ncols], s_bk[:])

                # x_conv half of xo -> s_xop (bias included, no mask yet)
                s_xconv = [pp.tile([P, WIN_OUT * W], BF16, tag=f"xconv{i}", name=f"s_xconv{i}") for i in range(2)]
                for i in range(2):
                    nc.sync.dma_start(s_xconv[i][:], t_xconv[i])
                s_wo1bt = pp.tile([P, 2, C], BF16, tag="wo1bt", name="s_wo1bt")
                nc.scalar.dma_start(s_wo1bt[:], t_wo1bt[:])
                s_trt = pp.tile([P, 2, 18, P], BF16, tag="trt", name="s_trt")
                for o in range(2):
                    nc.sync.dma_start(s_trt[:, o], t_trt[:, o])
                for o in range(2):
                    # right pad column must be zero (memset can't encode f32r);
                    # strided DMA would be 33x128 4-byte descriptors - use DVE
                    nc.vector.tensor_copy(s_xopad[o][:, :, W:W + 1], s_zcol[:, :, None])

                for o in range(2):
                    for (a0, nr) in XOP_CHUNKS:
                        c0, ncols = a0 * W, nr * W
                        ps = psMM.tile([P, 512], F32, tag="mm", name="ps_mm")
                        for i in range(2):
                            nc.tensor.matmul(
                                ps[:, :ncols],
                                s_wo1bt[:, i, o * CH:(o + 1) * CH],
                                s_xconv[i][:, c0:c0 + ncols],
                                start=(i == 0), stop=(i == 1),
                            )
                        nc.any.tensor_scalar_add(
                            s_xop[o][:, a0:a0 + nr, :],
                            ps[:, :ncols].rearrange("p (a w) -> p a w", w=W),
                            s_bxo[:, o:o + 1],
                        )

                # ---- QT = (Rm @ (A_dup? + A_own + A_peer))^T ----
                ps_qt = psQV.tile([P, 2 * CH], F32, tag="qv", name="ps_qt")
                if D:
                    # evacuate the local dup Gram during the exchange flight
                    nc.any.tensor_copy(s_A[:], ps_Adup[:, :CH])
                    nc.tensor.matmul(ps_qt[:], s_A[:], s_rt[:], start=True, stop=False)
                    nc.tensor.matmul(ps_qt[:], s_Asend[:], s_rt[:], start=False, stop=False)
                else:
                    nc.tensor.matmul(ps_qt[:], s_Asend[:], s_rt[:], start=True, stop=False)
                ins_qt_recv = nc.tensor.matmul(
                    ps_qt[:], s_Arecv[:], s_rt[:], start=False, stop=True
                ).ins
                nc.any.tensor_copy(s_QT[:], ps_qt[:])

            # ---- phase 2: attn half of xo + convT with a deeper PSUM pool ----
            with tc.tile_pool(name="psMM2", bufs=6, space="PSUM") as psMM2:
                for o in range(2):
                    for (a0, nr) in XO_CHUNKS:
                        c0, ncols = a0 * W, nr * W
                        ps = psMM2.tile([P, 512], F32, tag="mm2", name="ps_mm2")
                        nc.tensor.matmul(
                            ps[:, :ncols],
                            s_QT[:, o * CH:(o + 1) * CH],
                            s_k[:, c0:c0 + ncols],
                            start=True, stop=True,
                        )
                        dst = s_xopad[o][:, a0:a0 + nr, 0:W]
                        src2 = ps[:, :ncols].rearrange("p (a w) -> p a w", w=W)
                        nc.any.tensor_tensor(dst, src2, s_xop[o][:, a0:a0 + nr, :], mybir.AluOpType.add)
                        if a0 + nr == WIN_OUT:
                            # halo row (row 32) is zero on the bottom-half core
                            hd = s_xopad[o][:, WIN_OUT - 1:WIN_OUT, 0:W]
                            nc.any.tensor_scalar(
                                hd, hd, s_lastmask[:], None,
                                op0=mybir.AluOpType.mult,
                            )

                # ---- convT: 4 parity grids over local a in [0, 32) ----
                for r in range(2):
                    for a0 in (0, 8, 16, 24):
                        for o in range(2):
                            line = linep.tile([P, 8, 2 * W], F32, tag="line", name="line")
                            for s in range(2):
                                taps = CT_TAPS[(r, s)]
                                ps = psMM2.tile([P, 512], F32, tag="mm2", name="ps_mm2")
                                n_mm = len(taps) * 2
                                mi = 0
                                for (ky, kx, da, db) in taps:
                                    t = ky * 3 + kx
                                    for i in range(2):
                                        rhs = s_xopad[i][:, a0 + da:a0 + da + 8, db:db + W]
                                        nc.tensor.matmul(
                                            ps[:],
                                            s_trt[:, o, t * 2 + i, :],
                                            rhs,
                                            start=(mi == 0),
                                            stop=(mi == n_mm - 1),
                                        )
                                        mi += 1
                                nc.any.tensor_scalar_add(
                                    line[:, :, s::2],
                                    ps.rearrange("p (a w) -> p a w", w=W),
                                    s_trb[:, o:o + 1],
                                )
                            e0, e1 = ((nc.sync, nc.gpsimd), (nc.gpsimd, nc.scalar),
                                      (nc.scalar, nc.sync))[(r * 8 + a0 // 8 * 2 + o) % 3]
                            e0.dma_start(t_out[o, :, a0:a0 + 4, r, :], line[:, 0:4])
                            e1.dma_start(t_out[o, :, a0 + 4:a0 + 8, r, :], line[:, 4:8])

    # Splice in the two externally-incremented sem waits the Tile sim could
    # not model: (a) gate the SWDGE trigger on the kernel-entry barrier so the
    # peer has cleared its sems before our remote write+sem-inc arrives;
    # (b) gate the QT matmul that reads s_Arecv on the peer's data landing
    # (remote_sem += 2, one per DMA lane of slot 0).
    nc._bir_kernel_barrier_sem_replica_groups.extend(
        set(g) for g in [[0, 1], [2, 3], [4, 5], [6, 7]]
    )
    w_bar = nc.gpsimd.wait_ge(
        nc._bir_kernel_barrier_sem, nc.bir_kernel_barrier_sem_inc
    ).ins
    w_recv = nc.tensor.wait_ge(recv_sem, 2).ins

    def _move_before(wait_ins, target_ins):
        blocks = nc.main_func.blocks
        for b in blocks:
            if wait_ins in b.instructions:
                b.instructions.remove(wait_ins)
                break
        for b in blocks:
            if target_ins in b.instructions:
                idx = b.instructions.index(target_ins)
                b.instructions.insert(idx, wait_ins)
                return
        raise RuntimeError("target instruction not found in any block")

    _move_before(w_bar, ins_trigger)
    _move_before(w_recv, ins_qt_recv)

    nc.compile()
    return nc


def _ensure_ntff_hook():
    """antenv.axon_hooks is absent in this image; recreate it + install the
    ctypes NTFF hook so run_bass_kernel_spmd(trace=True) can profile."""
    try:
        from antenv import axon_hooks  # noqa: F401
        return
    except ImportError:
        pass
    try:
        import types
        import antenv
        mod = types.ModuleType("antenv.axon_hooks")
        _hook = [None]
        mod.set_axon_ntff_profile_hook = lambda h: _hook.__setitem__(0, h)
        mod.get_axon_ntff_profile_hook = lambda: _hook[0]
        sys.modules["antenv.axon_hooks"] = mod
        antenv.axon_hooks = mod
        from trn_agent_boot.trn_boot import _ntff_profile_via_ctypes
        mod.set_axon_ntff_profile_hook(
            _ntff_profile_via_ctypes("/opt/axon/libaxon_pjrt.so")
        )
    except Exception:
        pass


def kernel(**inputs):
    global LAST_EXEC_TIME_NS, LAST_PROFILE
    if "nc" not in _CACHE:
        _CACHE["nc"] = _build_program()
    nc = _CACHE["nc"]
    shared = _prep_weights(inputs)
    in_maps = _prep_core_inputs(inputs, shared)
    trace = os.environ.get("KERNEL_PROFILE", "") in ("1", "true")
    if trace:
        _ensure_ntff_hook()
    res = run_bass_kernel_spmd(nc, in_maps, core_ids=list(range(8)), trace=trace)
    LAST_EXEC_TIME_NS = getattr(res, "exec_time_ns", None)
    LAST_PROFILE = getattr(res, "profile_json", None)
    out = np.zeros((B, C, 2 * H, 2 * W), np.float32)
    for core in range(8):
        b, h = divmod(core, 2)
        o = res.results[core]["out"]  # (2, 128, 32, 2, 128)
        out[b, :, 64 * h:64 * (h + 1), :] = o.reshape(C, 64, 2 * W)
    return out


if __name__ == "__main__":
    print("smoke build only")
    _build_program()
    print("build ok")

